# revision 32
# baseline (speedup 1.0000x reference)
"""TRN2 Bass kernel for nn_DSSMEmbed (vq_codebook), v2.

Strategy (8 NeuronCores, data-parallel over batch, 256 imgs/core):
  - Activation layout: partitions = (x, channel), free = (y, img).
  - 3x3 convs as Toeplitz matmuls over 4-x windows (8 64x32 PE tiles,
    PSUM-bank-interleaved issue order); dy via PSUM accumulation at
    shifted free-dim (y) offsets with boundary clipping.
  - Tower2 (feeds the VQ argmax, precision-critical): convs in fp32
    (K<=57 fp32 runs dual-plane LOW_HIGH near bf16 rate; K=64 pays 2x),
    linear + everything after in FP32R (12-bit-mantissa operands,
    fp32 accumulate) — measured 0 argmax flips vs full fp32, ~2.2x
    faster than fp32 on K=128 matmuls.  Tower1 + final BxB in bf16.
  - Window tensors are FULL-Y (128, 16, Bloc) built with fat contiguous
    copies on the gpsimd dyndma engines (sync/scalar DIRECT2D DMA is
    engine-blocking and ~5x slower for SBUF->SBUF); t2 emb windows ship
    pre-windowed int8 from host (gpsimd casting DMA at t=0), t1 emb
    windows ship as bf16 and prefetch on the sync engine during t2lin.
    A host-shipped ones-row folds the emb bias into the dy=0 operator.
  - PSUM evacuations alternate scalar.activation / DVE ops.
  - Linear: K=8192 accumulation, lw streamed on the gpsimd DMA queue
    (starts draining at kernel t=0, deep buffer pool); lw2 is
    pre-rounded to the FP32R grid on host.
  - VQ: scores via PE (fp32), DVE max/max_index, AllGather of the 256
    u32 indices only (1KB); every core then gathers all 2048 codebook
    rows locally (bf16 indirect DMA) and PE-transposes them for the
    final matmul.  No 2MB collective on the critical path.
  - embed1 norms via DVE on (img, E); 1/(|e|+eps) and exp(scale) folded
    into the final evacuation as per-partition scalars.
"""
import sys

sys.path.insert(0, "/opt/trn_rl_repo")

import numpy as np
import concourse.bass as bass
import concourse.bacc as bacc
import concourse.mybir as mybir
import concourse.tile as tile
from concourse.bass_utils import run_bass_kernel_spmd

F32 = mybir.dt.float32
F32R = mybir.dt.float32r
BF16 = mybir.dt.bfloat16
U32 = mybir.dt.uint32
AF = mybir.ActivationFunctionType


def round_f32r(x):
    """Round fp32 array to the FP32R grid (12-bit mantissa, RNE-ish like
    libwalrus fp32_to_fp32r)."""
    u = np.ascontiguousarray(x, np.float32).view(np.uint32)
    u = (u + np.uint32(0x800)) & np.uint32(0xFFFFF000)
    return u.view(np.float32).copy()

NCORES = 8
B = 2048
BL = B // NCORES          # 256 imgs per core
H = W = 16
DICT, SE, CE, ESZ, NZ = 14, 8, 16, 512, 512
EPS = 1e-4
YB = H * BL               # free dim (y, img) = 4096

BO8 = [0, 4, 1, 5, 2, 6, 3, 7]      # PSUM-bank-interleaved b order

# ---------------------------------------------------------------------------
# host-side preprocessing
# ---------------------------------------------------------------------------


def make_windowed_oh_full(nat, ones_row):
    """nat: (DICT, H, W, Bloc) -> (4, 128, H, Bloc) int8 full-y windows.

    Window tensor t holds block b=t at rows 0.. and b=t+4 at rows 64..;
    rows h*64 + w*14 + d for window x' = 2b-1+w, w in 0..3.  Row h*64+56
    is a constant-one row (bias rides the dy=0 operator) when ones_row.
    """
    out = np.zeros((4, 128, H, nat.shape[-1]), dtype=np.int8)
    for b in range(8):
        t, h = b % 4, b // 4
        for w in range(4):
            xs = 2 * b - 1 + w
            if 0 <= xs < W:
                out[t, h * 64 + w * DICT:h * 64 + (w + 1) * DICT] = \
                    nat[:, :, xs, :]
        if ones_row:
            out[t, h * 64 + 56] = 1
    return out


def op_emb_win(wfold, bias):
    """Folded emb conv operator for 64x32 windowed scheme: (3, 4, 128, 32).

    wfold: (C_out=16, DICT, 3, 3).  lhsT[dy, t, h*64 + w*14 + d,
    xr*16 + co] = wfold[co, d, dy, w - xr] (dx = w - xr in 0..2).
    Row h*64+56 of the dy=0 operator carries the bias (ones-row input).
    """
    op = np.zeros((3, 4, 128, 32), dtype=np.float32)
    for dy in range(3):
        blk = np.zeros((57, 32), dtype=np.float32)
        for w in range(4):
            for xr in range(2):
                dx = w - xr
                if 0 <= dx <= 2:
                    blk[w * DICT:(w + 1) * DICT, xr * 16:(xr + 1) * 16] = \
                        wfold[:, :, dy, dx].T
        if dy == 1 and bias is not None:
            blk[56, 0:16] = bias
            blk[56, 16:32] = bias
        for h in range(2):
            op[dy, :, h * 64:h * 64 + 57, :] = blk[None]
    return op


def op_conv_win(wc, c_in, c_out):
    """Windowed 64-row conv operator: (3, 4, 128, px*c_out) with px=2.

    wc: (c_out, c_in, 3, 3).  Tensor t serves blocks b=t (rows 0..) and
    b=t+4 (rows 64..); rows w*c_in+ci for window x' = 2b-1+w (w in 0..3),
    cols xr*c_out+co.  Boundary rows (x'=-1 for b=0, x'=16 for b=7) are
    zeroed (window tensors hold zeros there anyway).
    """
    M = 2 * c_out
    op = np.zeros((3, 4, 128, M), dtype=np.float32)
    blk = np.zeros((4 * c_in, M), dtype=np.float32)
    for dy in range(3):
        blk[:] = 0.0
        for w in range(4):
            for xr in range(2):
                dx = w - xr
                if 0 <= dx <= 2:
                    blk[w * c_in:(w + 1) * c_in, xr * c_out:(xr + 1) * c_out] = \
                        wc[:, :, dy, dx].T
        for h in range(2):
            op[dy, :, h * 64:h * 64 + 4 * c_in, :] = blk[None]
        op[dy, 0, 0:c_in, :] = 0.0                    # b=0, w=0 (x'=-1)
        op[dy, 3, 64 + 3 * c_in:64 + 4 * c_in, :] = 0.0  # b=7, w=3 (x'=16)
    return op


def host_prep(inputs):
    s = np.asarray(inputs["s"])
    sp = np.asarray(inputs["s_prime"])
    se_w = np.asarray(inputs["state_embed"], dtype=np.float32)
    norms = np.sqrt((se_w * se_w).sum(1, keepdims=True))
    table = se_w / np.maximum(norms, 1.0)

    oh_s = (np.arange(DICT)[:, None, None, None] ==
            s.transpose(1, 2, 0)[None]).astype(np.int8)
    oh_sp = (np.arange(DICT)[:, None, None, None] ==
             sp.transpose(1, 2, 0)[None]).astype(np.int8)
    oh_d = oh_sp - oh_s

    emb_fold = np.einsum("oikl,di->odkl",
                         np.asarray(inputs["conv_embed_w"], np.float32), table)

    b_emb = np.asarray(inputs["conv_embed_b"], np.float32)
    shared = {
        "op_embt1": op_emb_win(emb_fold, b_emb),
        "op_embt2": op_emb_win(emb_fold, None),
        "op_c1t1": op_conv_win(np.asarray(inputs["p1c1_w"], np.float32), 16, 16),
        "op_c1t2": op_conv_win(np.asarray(inputs["p2c1_w"], np.float32), 16, 16),
        "op_c2t1": op_conv_win(np.asarray(inputs["p1c2_w"], np.float32), 16, 32),
        "op_c2t2": op_conv_win(np.asarray(inputs["p2c2_w"], np.float32), 16, 32),
    }

    def reorder_lin(lw):
        # K order: (chunk c, y, row r), r = xr*32+ch, x = c*4+xr
        lw = np.asarray(lw, np.float32).reshape(ESZ, 32, H, W)
        lw = lw.transpose(3, 1, 2, 0).reshape(4, 4, 32, H, ESZ)  # (c,xr,ch,y,E)
        return np.ascontiguousarray(
            lw.transpose(0, 3, 1, 2, 4).reshape(4, H, 128, ESZ).reshape(64, 128, ESZ))

    shared["lw_t1"] = reorder_lin(inputs["p1l_w"])
    shared["lw_t2"] = reorder_lin(inputs["p2l_w"])

    zv = np.asarray(inputs["z_vectors"], np.float32)
    zn = zv / np.sqrt((zv * zv).sum(1, keepdims=True))
    shared["znT"] = np.ascontiguousarray(zn.T)
    shared["znb"] = zn  # cast to bf16 in make_in_maps

    def conv_bias(bvec, c_out):
        reps = 128 // c_out
        return np.ascontiguousarray(
            np.tile(np.asarray(bvec, np.float32), reps)[:, None])

    # conv biases applied at evacuation (scalar activation bias); the DVE
    # half of the evacs needs them to be zero (true for this model).
    shared["b_c1t1"] = conv_bias(inputs["p1c1_b"], 16)
    shared["b_c1t2"] = conv_bias(inputs["p2c1_b"], 16)
    shared["b_c2t1"] = conv_bias(inputs["p1c2_b"], 32)
    shared["b_c2t2"] = conv_bias(inputs["p2c2_b"], 32)
    zero_bias = all(
        np.allclose(np.asarray(inputs[k], np.float32), 0.0)
        for k in ("p1c1_b", "p2c1_b", "p1c2_b", "p2c2_b"))
    shared["b_l1"] = np.ascontiguousarray(
        np.asarray(inputs["p1l_b"], np.float32).reshape(1, ESZ))
    shared["b_l2"] = np.ascontiguousarray(
        np.asarray(inputs["p2l_b"], np.float32).reshape(1, ESZ))

    esc = float(np.exp(np.asarray(inputs["scale"], np.float32).reshape(-1)[0]))

    percore = []
    for c in range(NCORES):
        sl = slice(c * BL, (c + 1) * BL)
        percore.append({
            "ohs": make_windowed_oh_full(oh_s[..., sl], True),
            "ohd": make_windowed_oh_full(oh_d[..., sl], False),
        })
    return shared, percore, esc, zero_bias


# ---------------------------------------------------------------------------
# device program
# ---------------------------------------------------------------------------


def _clip_dy(y0, ny, dy):
    s = max(y0, -dy)
    e = min(y0 + ny, H - dy)
    if s >= e:
        return None
    return (s - y0) * BL, (e - s) * BL, s + dy


def build_program(esc, zero_bias):
    from contextlib import ExitStack
    nc = bacc.Bacc("TRN2", target_bir_lowering=False, debug=False,
                   num_devices=NCORES)

    def din(name, shape, dt):
        return nc.dram_tensor(name, list(shape), dt, kind="ExternalInput").ap()

    ohs_d = din("ohs", (4, 128, H, BL), BF16)
    ohd_d = din("ohd", (4, 128, H, BL), mybir.dt.int8)
    op_embt1_d = din("op_embt1", (3, 4, 128, 32), BF16)
    op_embt2_d = din("op_embt2", (3, 4, 128, 32), F32)
    op_c1t1_d = din("op_c1t1", (3, 4, 128, 32), BF16)
    op_c1t2_d = din("op_c1t2", (3, 4, 128, 32), F32)
    op_c2t1_d = din("op_c2t1", (3, 4, 128, 64), BF16)
    op_c2t2_d = din("op_c2t2", (3, 4, 128, 64), F32)
    lw1_d = din("lw1", (64, 128, ESZ), BF16)
    lw2_d = din("lw2", (64, 128, ESZ), F32R)
    b_c1t1_d = din("b_c1t1", (128, 1), F32)
    b_c1t2_d = din("b_c1t2", (128, 1), F32)
    b_c2t1_d = din("b_c2t1", (128, 1), F32)
    b_c2t2_d = din("b_c2t2", (128, 1), F32)
    b_l1_d = din("b_l1", (1, ESZ), F32)
    b_l2_d = din("b_l2", (1, ESZ), F32)
    znt_d = din("znt", (ESZ, NZ), F32)
    zn_d = din("zn", (NZ, ESZ), F32)
    ident_d = din("ident", (128, 128), F32)

    out_d = nc.dram_tensor("out", [BL, B], F32, kind="ExternalOutput").ap()
    idx_d = nc.dram_tensor("idxl", [2, 128], U32).ap()
    idxg_d = nc.dram_tensor("idxg", [2 * NCORES, 128], U32,
                            addr_space="Shared").ap()

    with tile.TileContext(nc) as tc, ExitStack() as ES:
        cst = ES.enter_context(tc.tile_pool(name="cst", bufs=1))
        npool = ES.enter_context(tc.tile_pool(name="nat", bufs=1))
        epool = ES.enter_context(tc.tile_pool(name="emb", bufs=1))
        wpool = ES.enter_context(tc.tile_pool(name="win", bufs=1))

        ident_sb = cst.tile([128, 128], F32, tag="ident", name="ident")
        nc.sync.dma_start(ident_sb[:], ident_d[:])
        bias_sb = {}
        for nm, d in [("b_c1t1", b_c1t1_d), ("b_c1t2", b_c1t2_d),
                      ("b_c2t1", b_c2t1_d), ("b_c2t2", b_c2t2_d)]:
            t = cst.tile([128, 1], F32, tag=nm, name=nm)
            nc.sync.dma_start(t[:], d[:])
            bias_sb[nm] = t
        bl_sb = {}
        for nm, d in [("b_l1", b_l1_d), ("b_l2", b_l2_d)]:
            t = cst.tile([1, ESZ], F32, tag=f"{nm}r", name=f"{nm}r")
            nc.sync.dma_start(t[:], d[:])
            bl_sb[nm] = t
        ones_k = cst.tile([1, 128], F32, tag="ones_k", name="ones_k")
        nc.vector.memset(ones_k[:], 1.0)

        def load_ops(op_d, dt, width, pfx):
            ops = [[cst.tile([128, width], dt, tag=f"{pfx}{dy}{t}",
                             name=f"{pfx}{dy}{t}") for t in range(4)]
                   for dy in range(3)]
            for dy in range(3):
                for t in range(4):
                    nc.sync.dma_start(ops[dy][t][:], op_d[dy, t])
            return ops

        ops_embt2 = load_ops(op_embt2_d, F32, 32, "oe2")
        ops_embt1 = load_ops(op_embt1_d, BF16, 32, "oe1")
        ops_c1t2 = load_ops(op_c1t2_d, F32, 32, "oc12")
        ops_c1t1 = load_ops(op_c1t1_d, BF16, 32, "oc11")
        ops_c2t2 = load_ops(op_c2t2_d, F32, 64, "od12")
        ops_c2t1 = load_ops(op_c2t1_d, BF16, 64, "od11")

        # full-y window tiles: ONE physical set (F32-sized tags), used as
        # F32 by tower2 and rebound as BF16 by tower1 (same buffers).
        def win_set(dt):
            return [wpool.tile([128, H, BL], dt, tag=f"Wf{t}", name=f"W{t}")
                    for t in range(4)]

        winf = win_set(F32)
        # persistent zero rows at the image x-borders (x'=-1 / x'=16):
        # zero bytes are valid zeros for both dtypes.
        nc.vector.memset(winf[0][0:32], 0.0)
        nc.vector.memset(winf[3][96:128], 0.0)

        ev_ctr = [0]

        def evac(dst, ps_ap, kind, bias=None):
            """PSUM->SBUF evacuation alternating scalar / DVE."""
            use_dve = (ev_ctr[0] % 2 == 1) and (kind == "id" or zero_bias)
            ev_ctr[0] += 1
            if use_dve:
                if kind == "relu":
                    nc.vector.tensor_scalar_max(dst, ps_ap, 0.0)
                else:
                    nc.vector.tensor_copy(dst, ps_ap)
            else:
                af = AF.Relu if kind == "relu" else AF.Identity
                bb = bias[:] if bias is not None else 0.0
                nc.scalar.activation(dst, ps_ap, af, bias=bb)

        # ---------------- emb conv (windows DMA'd from DRAM) --------------
        def emb_conv(oh_d, ops, dt, tags, wins, load=True):
            outs = [npool.tile([128, YB], dt, tag=tg, name=tg) for tg in tags]
            if load:
                # int8->dt casting DMA on gpsimd dyndma; y-split so the
                # first ygroups' matmuls start after half the load.
                for yh in range(2):
                    for t in range(4):
                        nc.gpsimd.dma_start(
                            wins[t][:, yh * 8:yh * 8 + 8, :],
                            oh_d[t][:, yh * 8:yh * 8 + 8, :])
            with tc.tile_pool(name=f"ep{tags[0]}", bufs=3, space="PSUM") as pp:
                for yg in range(8):
                    y0 = 2 * yg
                    ps = [pp.tile([128, 2 * BL], F32, tag=f"p{i}",
                                  name=f"p{i}") for i in range(2)]
                    first = True
                    for dy in (0, -1, 1):
                        n0, N, ysrc = _clip_dy(y0, 2, dy)
                        nys = N // BL
                        for b in BO8:
                            t, hh = b % 4, b // 4
                            nc.tensor.matmul(
                                ps[hh][32 * t:32 * t + 32, n0:n0 + N],
                                ops[dy + 1][t][hh * 64:hh * 64 + 57, :],
                                wins[t][hh * 64:hh * 64 + 57,
                                        ysrc:ysrc + nys, :],
                                start=first, stop=(dy == 1),
                                tile_position=(hh * 64, 32 * t))
                        first = False
                    sl = slice(y0 * BL, (y0 + 2) * BL)
                    for i in range(2):
                        evac(outs[i][:, sl], ps[i][:], "id")
            return outs

        # -------- full-y window builder: 2-chunk natural -> 4 win tensors --
        def build_wins_full(nat2, wins):
            """Window tensor t rows [h*64 + (x'-(2b-1))*16 + ci], b=t+4h,
            full y; gpsimd dyndma only (sync/scalar DIRECT2D is ~5x
            slower for SBUF->SBUF).  Each copy is split into y-halves,
            first halves emitted first: they only depend on the producer
            evacuating ygroups 0-3, so the consumer's early matmuls can
            start while the producer's tail is still running."""
            for yh in range(2):
                for b in [0, 4, 1, 5, 2, 6, 3, 7]:
                    t, hh = b % 4, b // 4
                    x0 = 2 * b - 1
                    xs_s, xs_e = max(0, x0), min(W, x0 + 4)
                    if xs_s < 8 < xs_e:
                        pieces = [(xs_s, 8), (8, xs_e)]
                    else:
                        pieces = [(xs_s, xs_e)]
                    for (a, bb) in pieces:
                        ch = a // 8
                        src = nat2[ch][(a % 8) * 16:(a % 8) * 16 +
                                       (bb - a) * 16, :]
                        src3 = src.rearrange("p (y i) -> p y i", y=H)
                        nc.gpsimd.dma_start(
                            wins[t][hh * 64 + (a - x0) * 16:
                                    hh * 64 + (bb - x0) * 16,
                                    yh * 8:yh * 8 + 8, :],
                            src3[:, yh * 8:yh * 8 + 8, :])
            return wins

        # ---------------- c1 conv (64x32 8-tile, windowed) -----------------
        def c1_conv(ins2, ops, dt, bias, tags, wins):
            outs = [npool.tile([128, YB], dt, tag=tg, name=tg) for tg in tags]
            build_wins_full(ins2, wins)
            with tc.tile_pool(name=f"cp{tags[0]}", bufs=3, space="PSUM") as pp:
                for yg in range(8):
                    y0 = 2 * yg
                    ps = [pp.tile([128, 2 * BL], F32, tag=f"p{i}",
                                  name=f"p{i}") for i in range(2)]
                    first = True
                    for dy in (0, -1, 1):
                        n0, N, ysrc = _clip_dy(y0, 2, dy)
                        nys = N // BL
                        for b in BO8:
                            t, hh = b % 4, b // 4
                            nc.tensor.matmul(
                                ps[hh][32 * t:32 * t + 32, n0:n0 + N],
                                ops[dy + 1][t][hh * 64:hh * 64 + 64, :],
                                wins[t][hh * 64:hh * 64 + 64,
                                        ysrc:ysrc + nys, :],
                                start=first, stop=(dy == 1),
                                tile_position=(hh * 64, 32 * t))
                        first = False
                    sl = slice(y0 * BL, (y0 + 2) * BL)
                    for i in range(2):
                        evac(outs[i][:, sl], ps[i][:], "relu", bias)
            return outs

        # ---------------- c2 conv (64x64 4-tile, windowed) -----------------
        def c2_conv(ins2, ops, dt, bias, tags, wins):
            outs = [npool.tile([128, YB], dt, tag=tg, name=tg) for tg in tags]
            BORD = [0, 2, 4, 6, 1, 3, 5, 7]
            build_wins_full(ins2, wins)
            with tc.tile_pool(name=f"dp{tags[0]}", bufs=2, space="PSUM") as pp:
                for yg in range(8):
                    y0 = 2 * yg
                    ps = [pp.tile([128, 2 * BL], F32, tag=f"p{i}",
                                  name=f"p{i}") for i in range(4)]
                    first = True
                    for dy in (0, -1, 1):
                        n0, N, ysrc = _clip_dy(y0, 2, dy)
                        nys = N // BL
                        for b in BORD:
                            t, hh = b % 4, b // 4
                            nc.tensor.matmul(
                                ps[b // 2][64 * (b % 2):64 * (b % 2) + 64,
                                           n0:n0 + N],
                                ops[dy + 1][t][hh * 64:hh * 64 + 64, :],
                                wins[t][hh * 64:hh * 64 + 64,
                                        ysrc:ysrc + nys, :],
                                start=first, stop=(dy == 1),
                                tile_position=(hh * 64, 64 * (b % 2)))
                        first = False
                    sl = slice(y0 * BL, (y0 + 2) * BL)
                    for i in range(4):
                        evac(outs[i][:, sl], ps[i][:], "relu", bias)
            return outs

        # ---------------- linear (M=img, N=E; returns (img, E) chunks) ----
        def linear(c2o, lw_d, dt, bias_row, tagp, nbufs=8):
            embT = [epool.tile([128, ESZ], F32, tag=f"{tagp}T{m}",
                               name=f"{tagp}T{m}") for m in range(2)]
            with tc.tile_pool(name=f"lw{tagp}", bufs=nbufs) as lwp, \
                 tc.tile_pool(name=f"lp{tagp}", bufs=1, space="PSUM") as pp:
                ps = [pp.tile([128, ESZ], F32, tag=f"p{m}", name=f"p{m}")
                      for m in range(2)]
                for k in range(64):
                    lwt = lwp.tile([128, ESZ], dt, tag="lw", name="lw")
                    nc.gpsimd.dma_start(lwt[:], lw_d[k])
                    cch, y = k // 16, k % 16
                    for m in range(2):
                        lhsT = c2o[cch][:, y * BL + 128 * m:y * BL + 128 * m + 128]
                        nc.tensor.matmul(ps[m][:], lhsT, lwt[:],
                                         start=(k == 0), stop=False)
                for m in range(2):
                    nc.tensor.matmul(ps[m][:], ones_k[:],
                                     bias_row[:], start=False, stop=True)
                for m in range(2):
                    nc.scalar.activation(embT[m][:], ps[m][:], AF.Identity)
            return embT

        def transpose_back(embT, dt, tagp):
            """(img,E) 2 chunks -> (E,img) 4 chunks of dtype dt."""
            emb = [epool.tile([128, BL], dt, tag=f"{tagp}{e}", name=f"{tagp}{e}")
                   for e in range(4)]
            with tc.tile_pool(name=f"tp{tagp}", bufs=2, space="PSUM") as tpp:
                for m in range(2):
                    for e in range(4):
                        tp = tpp.tile([128, 128], F32, tag="tp", name="tp")
                        nc.tensor.transpose(tp[:], embT[m][:, 128 * e:128 * e + 128],
                                            ident_sb[:])
                        nc.vector.tensor_copy(emb[e][:, 128 * m:128 * m + 128],
                                              tp[:])
            return emb

        # ================== tower 2 (fp32 delta path) ==================
        with nc.named_scope("t2emb"):
            d3 = emb_conv(ohd_d, ops_embt2, F32, ["A0", "A1"], winf)
        with nc.named_scope("t2c1"):
            c1o2 = c1_conv(d3, ops_c1t2, F32, bias_sb["b_c1t2"],
                           ["B0", "B1"], winf)
        with nc.named_scope("t2c2"):
            c2o2 = c2_conv(c1o2, ops_c2t2, F32R, bias_sb["b_c2t2"],
                           ["C0", "C1", "A0", "A1"], winf)
        # tower1 emb windows: prefetch on the sync engine while t2lin
        # runs; WAR on the shared window buffers delays this to t2c2-end.
        winb = win_set(BF16)
        for t in range(4):
            nc.sync.dma_start(winb[t][:], ohs_d[t])

        with nc.named_scope("t2lin"):
            embT2 = linear(c2o2, lw2_d, F32R, bl_sb["b_l2"], "e2")
            embed2 = transpose_back(embT2, F32, "e2n")

        # ================== VQ scores + idx AllGather ==================
        with nc.named_scope("vq"):
            with tc.tile_pool(name="vq", bufs=1) as vqp, \
                 tc.tile_pool(name="vqp", bufs=1, space="PSUM") as vpp:
                znt_sb = []
                for e in range(4):
                    t = vqp.tile([128, NZ], F32, tag=f"znt{e}", name=f"znt{e}")
                    nc.sync.dma_start(t[:], znt_d[128 * e:128 * e + 128, :])
                    znt_sb.append(t)
                sps = [vpp.tile([128, NZ], F32, tag=f"s{m}", name=f"s{m}")
                       for m in range(2)]
                for e in range(4):
                    for m in range(2):
                        nc.tensor.matmul(sps[m][:],
                                         embed2[e][:, 128 * m:128 * m + 128],
                                         znt_sb[e][:], start=(e == 0),
                                         stop=(e == 3))
                for m in range(2):
                    sc = vqp.tile([128, NZ], F32, tag=f"sc{m}", name=f"sc{m}")
                    nc.vector.tensor_copy(sc[:], sps[m][:])
                    mx = vqp.tile([128, 8], F32, tag=f"mx{m}", name=f"mx{m}")
                    nc.vector.max(mx[:], sc[:])
                    ix = vqp.tile([128, 8], U32, tag=f"ix{m}", name=f"ix{m}")
                    nc.vector.max_index(ix[:], mx[:], sc[:])
                    # sync engine: gated on the argmax, but nothing else
                    # needs sync at that point; keeps gpsimd queue moving.
                    nc.sync.dma_start(idx_d[m], ix[:, 0:1])

        # ================== tower 1 (bf16) ==================
        with nc.named_scope("t1emb"):
            se3 = emb_conv(ohs_d, ops_embt1, BF16, ["B0", "B1"], winb,
                           load=False)
        with nc.named_scope("t1c1"):
            c1o1 = c1_conv(se3, ops_c1t1, BF16, bias_sb["b_c1t1"],
                           ["C0", "C1"], winb)
        with nc.named_scope("t1c2"):
            c2o1 = c2_conv(c1o1, ops_c2t1, BF16, bias_sb["b_c2t1"],
                           ["A0", "A1", "B0", "B1"], winb)

        # AllGather the 2x128 u32 indices (1KB): triggered late so the
        # gpsimd queue never blocks on the argmax semaphore.
        nc.gpsimd.collective_compute(
            "AllGather", mybir.AluOpType.bypass,
            replica_groups=[list(range(NCORES))],
            ins=[idx_d[:]], outs=[idxg_d[:]])

        with nc.named_scope("t1lin"):
            embT1 = linear(c2o1, lw1_d, BF16, bl_sb["b_l1"], "e1", nbufs=16)
            e1b = transpose_back(embT1, BF16, "e1b")

        # ============== gather all-B codebook rows locally ==============
        # Emitted before t1lin: the gathers + PE transposes overlap the
        # t1 linear phase.  gsb_e reuses dead npool buffers (B*/C* are
        # free once t1c2's matmuls have consumed se3/c1o1).
        with nc.named_scope("zgat"):
            # two (128, 2B) bf16 tiles in the dead C0/C1 buffers; e-chunk
            # view: gsb_e[e] = gt[e//2][:, (e%2)*B : (e%2)*B+B]
            gt = [npool.tile([128, 2 * B], BF16, tag=tg, name=f"gt{j}")
                  for j, tg in enumerate(["C0", "C1"])]
            gsb_e = [gt[e // 2][:, (e % 2) * B:(e % 2) * B + B]
                     for e in range(4)]
            with tc.tile_pool(name="zg", bufs=4) as zgp, \
                 tc.tile_pool(name="zgp", bufs=4, space="PSUM") as zpp:
                gidx = epool.tile([128, 2 * NCORES], U32, tag="gidx",
                                  name="gidx")
                nc.sync.dma_start(gidx[:], idxg_d.rearrange("a p -> p a"))
                for a in range(2 * NCORES):
                    zlg = zgp.tile([128, ESZ], F32, tag="zl", name="zl")
                    nc.gpsimd.indirect_dma_start(
                        out=zlg[:], out_offset=None, in_=zn_d[:],
                        in_offset=bass.IndirectOffsetOnAxis(
                            ap=gidx[:, a:a + 1], axis=0))
                    for e in range(4):
                        tp = zpp.tile([128, 128], F32, tag="tp", name="tp")
                        nc.tensor.transpose(
                            tp[:], zlg[:, 128 * e:128 * e + 128],
                            ident_sb[:])
                        if (a + e) % 2:
                            nc.vector.tensor_copy(
                                gsb_e[e][:, 128 * a:128 * a + 128], tp[:])
                        else:
                            nc.scalar.activation(
                                gsb_e[e][:, 128 * a:128 * a + 128],
                                tp[:], AF.Identity)

        # ================== norms ==================
        with tc.tile_pool(name="nrm", bufs=1) as nrp:
            rnt = [epool.tile([128, 1], F32, tag=f"rnt{m}", name=f"rnt{m}")
                   for m in range(2)]
            for m in range(2):
                sq = nrp.tile([128, ESZ], F32, tag="sq", name="sq")
                nc.vector.tensor_mul(sq[:], embT1[m][:], embT1[m][:])
                n2 = nrp.tile([128, 1], F32, tag="n2", name="n2")
                nc.vector.tensor_reduce(n2[:], sq[:], mybir.AxisListType.X,
                                        mybir.AluOpType.add)
                nc.scalar.sqrt(n2[:], n2[:])
                nc.vector.tensor_scalar_add(n2[:], n2[:], EPS)
                nc.vector.reciprocal(n2[:], n2[:])
                nc.vector.tensor_scalar_mul(rnt[m][:], n2[:], esc)

        # ================== final (bf16) ==================
        with nc.named_scope("fin"):
            with tc.tile_pool(name="finp", bufs=2, space="PSUM") as fpp:
                osb = [npool.tile([128, B], F32, tag=tg, name=f"o{m}")
                       for m, tg in enumerate(["A0", "A1"])]
                for n in range(4):
                    for m in range(2):
                        fp = fpp.tile([128, 512], F32, tag=f"f{m}",
                                      name=f"f{m}")
                        for e in range(4):
                            nc.tensor.matmul(fp[:],
                                             e1b[e][:, 128 * m:128 * m + 128],
                                             gsb_e[e][:, 512 * n:512 * n + 512],
                                             start=(e == 0), stop=(e == 3))
                        nc.vector.tensor_scalar_mul(
                            osb[m][:, 512 * n:512 * n + 512], fp[:], rnt[m][:])
                for m in range(2):
                    nc.sync.dma_start(out_d[128 * m:128 * m + 128, :], osb[m][:])

    nc.compile()
    return nc


def make_in_maps(shared, percore):
    import ml_dtypes
    bf = ml_dtypes.bfloat16

    def b16(x):
        return np.asarray(x, np.float32).astype(bf)

    base = {
        "op_embt1": b16(shared["op_embt1"]),
        "op_embt2": np.ascontiguousarray(shared["op_embt2"], np.float32),
        "op_c1t1": b16(shared["op_c1t1"]),
        "op_c1t2": np.ascontiguousarray(shared["op_c1t2"], np.float32),
        "op_c2t1": b16(shared["op_c2t1"]),
        "op_c2t2": np.ascontiguousarray(shared["op_c2t2"], np.float32),
        "lw1": b16(shared["lw_t1"]),
        "lw2": round_f32r(shared["lw_t2"]),
        "b_c1t1": shared["b_c1t1"], "b_c1t2": shared["b_c1t2"],
        "b_c2t1": shared["b_c2t1"], "b_c2t2": shared["b_c2t2"],
        "b_l1": shared["b_l1"], "b_l2": shared["b_l2"],
        "znt": shared["znT"], "zn": np.ascontiguousarray(shared["znb"],
                                                         np.float32),
        "ident": np.eye(128, dtype=np.float32),
    }
    maps = []
    for pc in percore:
        m = dict(base)
        m["ohs"] = np.ascontiguousarray(pc["ohs"].astype(bf))
        m["ohd"] = np.ascontiguousarray(pc["ohd"])
        maps.append(m)
    return maps


def kernel(**inputs):
    dsf = np.asarray(inputs.get("downscale_factor", 1)).reshape(-1)
    dsf = int(dsf[0]) if dsf.size else 1
    assert dsf == 1, f"only downscale_factor=1 supported, got {dsf}"
    shared, percore, esc, zero_bias = host_prep(inputs)
    nc = build_program(esc, zero_bias)
    maps = make_in_maps(shared, percore)
    res = run_bass_kernel_spmd(nc, maps, list(range(NCORES)))
    out = np.concatenate([res.results[c]["out"] for c in range(NCORES)],
                         axis=0)
    return out.astype(np.float32)


def run_for_test(inputs, trace=False):
    """test.py hook: returns (out, BassKernelResults)."""
    shared, percore, esc, zero_bias = host_prep(inputs)
    nc = build_program(esc, zero_bias)
    maps = make_in_maps(shared, percore)
    res = run_bass_kernel_spmd(nc, maps, list(range(NCORES)), trace=trace)
    out = np.concatenate([res.results[c]["out"] for c in range(NCORES)],
                         axis=0)
    return out.astype(np.float32), res


# revision 33
# speedup vs baseline: 1.0616x; 1.0616x over previous
"""TRN2 Bass kernel for nn_DSSMEmbed (vq_codebook), v2.

Strategy (8 NeuronCores, data-parallel over batch, 256 imgs/core):
  - Activation layout: partitions = (x, channel), free = (y, img).
  - 3x3 convs as Toeplitz matmuls over 4-x windows (8 64x32 PE tiles,
    PSUM-bank-interleaved issue order); dy via PSUM accumulation at
    shifted free-dim (y) offsets with boundary clipping.
  - Tower2 (feeds the VQ argmax, precision-critical): convs in fp32
    (K<=57 fp32 runs dual-plane LOW_HIGH near bf16 rate; K=64 pays 2x),
    linear + everything after in FP32R (12-bit-mantissa operands,
    fp32 accumulate) — measured 0 argmax flips vs full fp32, ~2.2x
    faster than fp32 on K=128 matmuls.  Tower1 + final BxB in bf16.
  - Window tensors are FULL-Y (128, 16, Bloc) built with fat contiguous
    copies on the gpsimd dyndma engines (sync/scalar DIRECT2D DMA is
    engine-blocking and ~5x slower for SBUF->SBUF); t2 emb windows ship
    pre-windowed int8 from host (gpsimd casting DMA at t=0), t1 emb
    windows ship as bf16 and prefetch on the sync engine during t2lin.
    A host-shipped ones-row folds the emb bias into the dy=0 operator.
  - PSUM evacuations alternate scalar.activation / DVE ops.
  - Linear: K=8192 accumulation, lw streamed on the gpsimd DMA queue
    (starts draining at kernel t=0, deep buffer pool); lw2 is
    pre-rounded to the FP32R grid on host.
  - VQ: scores via PE (fp32), DVE max/max_index, AllGather of the 256
    u32 indices only (1KB); every core then gathers all 2048 codebook
    rows locally (bf16 indirect DMA) and PE-transposes them for the
    final matmul.  No 2MB collective on the critical path.
  - embed1 norms via DVE on (img, E); 1/(|e|+eps) and exp(scale) folded
    into the final evacuation as per-partition scalars.
"""
import sys

sys.path.insert(0, "/opt/trn_rl_repo")

import numpy as np
import concourse.bass as bass
import concourse.bacc as bacc
import concourse.mybir as mybir
import concourse.tile as tile
from concourse.bass_utils import run_bass_kernel_spmd

F32 = mybir.dt.float32
F32R = mybir.dt.float32r
BF16 = mybir.dt.bfloat16
U32 = mybir.dt.uint32
AF = mybir.ActivationFunctionType


def round_f32r(x):
    """Round fp32 array to the FP32R grid (12-bit mantissa, RNE-ish like
    libwalrus fp32_to_fp32r)."""
    u = np.ascontiguousarray(x, np.float32).view(np.uint32)
    u = (u + np.uint32(0x800)) & np.uint32(0xFFFFF000)
    return u.view(np.float32).copy()

NCORES = 8
B = 2048
BL = B // NCORES          # 256 imgs per core
H = W = 16
DICT, SE, CE, ESZ, NZ = 14, 8, 16, 512, 512
EPS = 1e-4
YB = H * BL               # free dim (y, img) = 4096

BO8 = [0, 4, 1, 5, 2, 6, 3, 7]      # PSUM-bank-interleaved b order

# ---------------------------------------------------------------------------
# host-side preprocessing
# ---------------------------------------------------------------------------


def make_windowed_oh_full(nat, ones_row):
    """nat: (DICT, H, W, Bloc) -> (4, 128, H, Bloc) int8 full-y windows.

    Window tensor t holds block b=t at rows 0.. and b=t+4 at rows 64..;
    rows h*64 + w*14 + d for window x' = 2b-1+w, w in 0..3.  Row h*64+56
    is a constant-one row (bias rides the dy=0 operator) when ones_row.
    """
    out = np.zeros((4, 128, H, nat.shape[-1]), dtype=np.int8)
    for b in range(8):
        t, h = b % 4, b // 4
        for w in range(4):
            xs = 2 * b - 1 + w
            if 0 <= xs < W:
                out[t, h * 64 + w * DICT:h * 64 + (w + 1) * DICT] = \
                    nat[:, :, xs, :]
        if ones_row:
            out[t, h * 64 + 56] = 1
    return out


def op_emb_win(wfold, bias):
    """Folded emb conv operator for 64x32 windowed scheme: (3, 4, 128, 32).

    wfold: (C_out=16, DICT, 3, 3).  lhsT[dy, t, h*64 + w*14 + d,
    xr*16 + co] = wfold[co, d, dy, w - xr] (dx = w - xr in 0..2).
    Row h*64+56 of the dy=0 operator carries the bias (ones-row input).
    """
    op = np.zeros((3, 4, 128, 32), dtype=np.float32)
    for dy in range(3):
        blk = np.zeros((57, 32), dtype=np.float32)
        for w in range(4):
            for xr in range(2):
                dx = w - xr
                if 0 <= dx <= 2:
                    blk[w * DICT:(w + 1) * DICT, xr * 16:(xr + 1) * 16] = \
                        wfold[:, :, dy, dx].T
        if dy == 1 and bias is not None:
            blk[56, 0:16] = bias
            blk[56, 16:32] = bias
        for h in range(2):
            op[dy, :, h * 64:h * 64 + 57, :] = blk[None]
    return op


def op_conv_win(wc, c_in, c_out):
    """Windowed 64-row conv operator: (3, 4, 128, px*c_out) with px=2.

    wc: (c_out, c_in, 3, 3).  Tensor t serves blocks b=t (rows 0..) and
    b=t+4 (rows 64..); rows w*c_in+ci for window x' = 2b-1+w (w in 0..3),
    cols xr*c_out+co.  Boundary rows (x'=-1 for b=0, x'=16 for b=7) are
    zeroed (window tensors hold zeros there anyway).
    """
    M = 2 * c_out
    op = np.zeros((3, 4, 128, M), dtype=np.float32)
    blk = np.zeros((4 * c_in, M), dtype=np.float32)
    for dy in range(3):
        blk[:] = 0.0
        for w in range(4):
            for xr in range(2):
                dx = w - xr
                if 0 <= dx <= 2:
                    blk[w * c_in:(w + 1) * c_in, xr * c_out:(xr + 1) * c_out] = \
                        wc[:, :, dy, dx].T
        for h in range(2):
            op[dy, :, h * 64:h * 64 + 4 * c_in, :] = blk[None]
        op[dy, 0, 0:c_in, :] = 0.0                    # b=0, w=0 (x'=-1)
        op[dy, 3, 64 + 3 * c_in:64 + 4 * c_in, :] = 0.0  # b=7, w=3 (x'=16)
    return op


def host_prep(inputs):
    s = np.asarray(inputs["s"])
    sp = np.asarray(inputs["s_prime"])
    se_w = np.asarray(inputs["state_embed"], dtype=np.float32)
    norms = np.sqrt((se_w * se_w).sum(1, keepdims=True))
    table = se_w / np.maximum(norms, 1.0)

    oh_s = (np.arange(DICT)[:, None, None, None] ==
            s.transpose(1, 2, 0)[None]).astype(np.int8)
    oh_sp = (np.arange(DICT)[:, None, None, None] ==
             sp.transpose(1, 2, 0)[None]).astype(np.int8)
    oh_d = oh_sp - oh_s

    emb_fold = np.einsum("oikl,di->odkl",
                         np.asarray(inputs["conv_embed_w"], np.float32), table)

    b_emb = np.asarray(inputs["conv_embed_b"], np.float32)
    shared = {
        "op_embt1": op_emb_win(emb_fold, b_emb),
        "op_embt2": op_emb_win(emb_fold, None),
        "op_c1t1": op_conv_win(np.asarray(inputs["p1c1_w"], np.float32), 16, 16),
        "op_c1t2": op_conv_win(np.asarray(inputs["p2c1_w"], np.float32), 16, 16),
        "op_c2t1": op_conv_win(np.asarray(inputs["p1c2_w"], np.float32), 16, 32),
        "op_c2t2": op_conv_win(np.asarray(inputs["p2c2_w"], np.float32), 16, 32),
    }

    def reorder_lin(lw):
        # K order: (chunk c, y, row r), r = xr*32+ch, x = c*4+xr
        lw = np.asarray(lw, np.float32).reshape(ESZ, 32, H, W)
        lw = lw.transpose(3, 1, 2, 0).reshape(4, 4, 32, H, ESZ)  # (c,xr,ch,y,E)
        return np.ascontiguousarray(
            lw.transpose(0, 3, 1, 2, 4).reshape(4, H, 128, ESZ).reshape(64, 128, ESZ))

    shared["lw_t1"] = reorder_lin(inputs["p1l_w"])
    shared["lw_t2"] = reorder_lin(inputs["p2l_w"])

    zv = np.asarray(inputs["z_vectors"], np.float32)
    zn = zv / np.sqrt((zv * zv).sum(1, keepdims=True))
    shared["znT"] = np.ascontiguousarray(zn.T)
    shared["znb"] = zn  # cast to bf16 in make_in_maps

    def conv_bias(bvec, c_out):
        reps = 128 // c_out
        return np.ascontiguousarray(
            np.tile(np.asarray(bvec, np.float32), reps)[:, None])

    # conv biases applied at evacuation (scalar activation bias); the DVE
    # half of the evacs needs them to be zero (true for this model).
    shared["b_c1t1"] = conv_bias(inputs["p1c1_b"], 16)
    shared["b_c1t2"] = conv_bias(inputs["p2c1_b"], 16)
    shared["b_c2t1"] = conv_bias(inputs["p1c2_b"], 32)
    shared["b_c2t2"] = conv_bias(inputs["p2c2_b"], 32)
    zero_bias = all(
        np.allclose(np.asarray(inputs[k], np.float32), 0.0)
        for k in ("p1c1_b", "p2c1_b", "p1c2_b", "p2c2_b"))
    shared["b_l1"] = np.ascontiguousarray(
        np.asarray(inputs["p1l_b"], np.float32).reshape(1, ESZ))
    shared["b_l2"] = np.ascontiguousarray(
        np.asarray(inputs["p2l_b"], np.float32).reshape(1, ESZ))

    esc = float(np.exp(np.asarray(inputs["scale"], np.float32).reshape(-1)[0]))

    percore = []
    for c in range(NCORES):
        sl = slice(c * BL, (c + 1) * BL)
        percore.append({
            "ohs": make_windowed_oh_full(oh_s[..., sl], True),
            "ohd": make_windowed_oh_full(oh_d[..., sl], False),
        })
    return shared, percore, esc, zero_bias


# ---------------------------------------------------------------------------
# device program
# ---------------------------------------------------------------------------


def _clip_dy(y0, ny, dy):
    s = max(y0, -dy)
    e = min(y0 + ny, H - dy)
    if s >= e:
        return None
    return (s - y0) * BL, (e - s) * BL, s + dy


def build_program(esc, zero_bias):
    from contextlib import ExitStack
    nc = bacc.Bacc("TRN2", target_bir_lowering=False, debug=False,
                   num_devices=NCORES)

    def din(name, shape, dt):
        return nc.dram_tensor(name, list(shape), dt, kind="ExternalInput").ap()

    ohs_d = din("ohs", (4, 128, H, BL), BF16)
    ohd_d = din("ohd", (4, 128, H, BL), mybir.dt.int8)
    op_embt1_d = din("op_embt1", (3, 4, 128, 32), BF16)
    op_embt2_d = din("op_embt2", (3, 4, 128, 32), F32)
    op_c1t1_d = din("op_c1t1", (3, 4, 128, 32), BF16)
    op_c1t2_d = din("op_c1t2", (3, 4, 128, 32), F32)
    op_c2t1_d = din("op_c2t1", (3, 4, 128, 64), BF16)
    op_c2t2_d = din("op_c2t2", (3, 4, 128, 64), F32)
    lw1_d = din("lw1", (64, 128, ESZ), BF16)
    lw2_d = din("lw2", (64, 128, ESZ), F32R)
    b_c1t1_d = din("b_c1t1", (128, 1), F32)
    b_c1t2_d = din("b_c1t2", (128, 1), F32)
    b_c2t1_d = din("b_c2t1", (128, 1), F32)
    b_c2t2_d = din("b_c2t2", (128, 1), F32)
    b_l1_d = din("b_l1", (1, ESZ), F32)
    b_l2_d = din("b_l2", (1, ESZ), F32)
    znt_d = din("znt", (ESZ, NZ), F32)
    zn_d = din("zn", (NZ, ESZ), F32)
    ident_d = din("ident", (128, 128), F32)

    out_d = nc.dram_tensor("out", [BL, B], F32, kind="ExternalOutput").ap()
    idx_d = nc.dram_tensor("idxl", [2, 128], U32).ap()
    idxg_d = nc.dram_tensor("idxg", [2 * NCORES, 128], U32,
                            addr_space="Shared").ap()

    with tile.TileContext(nc) as tc, ExitStack() as ES:
        cst = ES.enter_context(tc.tile_pool(name="cst", bufs=1))
        npool = ES.enter_context(tc.tile_pool(name="nat", bufs=1))
        epool = ES.enter_context(tc.tile_pool(name="emb", bufs=1))
        wpool = ES.enter_context(tc.tile_pool(name="win", bufs=1))

        ident_sb = cst.tile([128, 128], F32, tag="ident", name="ident")
        nc.sync.dma_start(ident_sb[:], ident_d[:])
        bias_sb = {}
        for nm, d in [("b_c1t1", b_c1t1_d), ("b_c1t2", b_c1t2_d),
                      ("b_c2t1", b_c2t1_d), ("b_c2t2", b_c2t2_d)]:
            t = cst.tile([128, 1], F32, tag=nm, name=nm)
            nc.sync.dma_start(t[:], d[:])
            bias_sb[nm] = t
        bl_sb = {}
        for nm, d in [("b_l1", b_l1_d), ("b_l2", b_l2_d)]:
            t = cst.tile([1, ESZ], F32, tag=f"{nm}r", name=f"{nm}r")
            nc.sync.dma_start(t[:], d[:])
            bl_sb[nm] = t
        ones_k = cst.tile([1, 128], F32, tag="ones_k", name="ones_k")
        nc.vector.memset(ones_k[:], 1.0)

        def load_ops(op_d, dt, width, pfx):
            ops = [[cst.tile([128, width], dt, tag=f"{pfx}{dy}{t}",
                             name=f"{pfx}{dy}{t}") for t in range(4)]
                   for dy in range(3)]
            for dy in range(3):
                for t in range(4):
                    nc.sync.dma_start(ops[dy][t][:], op_d[dy, t])
            return ops

        ops_embt2 = load_ops(op_embt2_d, F32, 32, "oe2")
        ops_embt1 = load_ops(op_embt1_d, BF16, 32, "oe1")
        ops_c1t2 = load_ops(op_c1t2_d, F32, 32, "oc12")
        ops_c1t1 = load_ops(op_c1t1_d, BF16, 32, "oc11")
        ops_c2t2 = load_ops(op_c2t2_d, F32, 64, "od12")
        ops_c2t1 = load_ops(op_c2t1_d, BF16, 64, "od11")

        # full-y window tiles: ONE physical set (F32-sized tags), used as
        # F32 by tower2 and rebound as BF16 by tower1 (same buffers).
        def win_set(dt):
            return [wpool.tile([128, H, BL], dt, tag=f"Wf{t}", name=f"W{t}")
                    for t in range(4)]

        winf = win_set(F32)
        # persistent zero rows at the image x-borders (x'=-1 / x'=16):
        # zero bytes are valid zeros for both dtypes.
        nc.vector.memset(winf[0][0:32], 0.0)
        nc.vector.memset(winf[3][96:128], 0.0)

        ev_ctr = [0]

        def evac(dst, ps_ap, kind, bias=None):
            """PSUM->SBUF evacuation alternating scalar / DVE."""
            use_dve = (ev_ctr[0] % 2 == 1) and (kind == "id" or zero_bias)
            ev_ctr[0] += 1
            if use_dve:
                if kind == "relu":
                    nc.vector.tensor_scalar_max(dst, ps_ap, 0.0)
                else:
                    nc.vector.tensor_copy(dst, ps_ap)
            else:
                af = AF.Relu if kind == "relu" else AF.Identity
                bb = bias[:] if bias is not None else 0.0
                nc.scalar.activation(dst, ps_ap, af, bias=bb)

        # ---------------- emb conv (windows DMA'd from DRAM) --------------
        def emb_conv(oh_d, ops, dt, tags, wins, load=True):
            outs = [npool.tile([128, YB], dt, tag=tg, name=tg) for tg in tags]
            if load:
                # int8->dt casting DMA on gpsimd dyndma; y-split so the
                # first ygroups' matmuls start after half the load.
                for yh in range(2):
                    for t in range(4):
                        nc.gpsimd.dma_start(
                            wins[t][:, yh * 8:yh * 8 + 8, :],
                            oh_d[t][:, yh * 8:yh * 8 + 8, :])
            with tc.tile_pool(name=f"ep{tags[0]}", bufs=3, space="PSUM") as pp:
                for yg in range(8):
                    y0 = 2 * yg
                    ps = [pp.tile([128, 2 * BL], F32, tag=f"p{i}",
                                  name=f"p{i}") for i in range(2)]
                    first = True
                    for dy in (0, -1, 1):
                        n0, N, ysrc = _clip_dy(y0, 2, dy)
                        nys = N // BL
                        for b in BO8:
                            t, hh = b % 4, b // 4
                            nc.tensor.matmul(
                                ps[hh][32 * t:32 * t + 32, n0:n0 + N],
                                ops[dy + 1][t][hh * 64:hh * 64 + 57, :],
                                wins[t][hh * 64:hh * 64 + 57,
                                        ysrc:ysrc + nys, :],
                                start=first, stop=(dy == 1),
                                tile_position=(hh * 64, 32 * t))
                        first = False
                    sl = slice(y0 * BL, (y0 + 2) * BL)
                    for i in range(2):
                        evac(outs[i][:, sl], ps[i][:], "id")
            return outs

        # -------- full-y window builder: 2-chunk natural -> 4 win tensors --
        def build_wins_full(nat2, wins):
            """Window tensor t rows [h*64 + (x'-(2b-1))*16 + ci], b=t+4h,
            full y; gpsimd dyndma only (sync/scalar DIRECT2D is ~5x
            slower for SBUF->SBUF).  Each copy is split into y-halves,
            first halves emitted first: they only depend on the producer
            evacuating ygroups 0-3, so the consumer's early matmuls can
            start while the producer's tail is still running."""
            for yh in range(2):
                for b in [0, 4, 1, 5, 2, 6, 3, 7]:
                    t, hh = b % 4, b // 4
                    x0 = 2 * b - 1
                    xs_s, xs_e = max(0, x0), min(W, x0 + 4)
                    if xs_s < 8 < xs_e:
                        pieces = [(xs_s, 8), (8, xs_e)]
                    else:
                        pieces = [(xs_s, xs_e)]
                    for (a, bb) in pieces:
                        ch = a // 8
                        src = nat2[ch][(a % 8) * 16:(a % 8) * 16 +
                                       (bb - a) * 16, :]
                        src3 = src.rearrange("p (y i) -> p y i", y=H)
                        nc.gpsimd.dma_start(
                            wins[t][hh * 64 + (a - x0) * 16:
                                    hh * 64 + (bb - x0) * 16,
                                    yh * 8:yh * 8 + 8, :],
                            src3[:, yh * 8:yh * 8 + 8, :])
            return wins

        # ---------------- c1 conv (64x32 8-tile, windowed) -----------------
        def c1_conv(ins2, ops, dt, bias, tags, wins):
            outs = [npool.tile([128, YB], dt, tag=tg, name=tg) for tg in tags]
            build_wins_full(ins2, wins)
            with tc.tile_pool(name=f"cp{tags[0]}", bufs=3, space="PSUM") as pp:
                for yg in range(8):
                    y0 = 2 * yg
                    ps = [pp.tile([128, 2 * BL], F32, tag=f"p{i}",
                                  name=f"p{i}") for i in range(2)]
                    first = True
                    for dy in (0, -1, 1):
                        n0, N, ysrc = _clip_dy(y0, 2, dy)
                        nys = N // BL
                        for b in BO8:
                            t, hh = b % 4, b // 4
                            nc.tensor.matmul(
                                ps[hh][32 * t:32 * t + 32, n0:n0 + N],
                                ops[dy + 1][t][hh * 64:hh * 64 + 64, :],
                                wins[t][hh * 64:hh * 64 + 64,
                                        ysrc:ysrc + nys, :],
                                start=first, stop=(dy == 1),
                                tile_position=(hh * 64, 32 * t))
                        first = False
                    sl = slice(y0 * BL, (y0 + 2) * BL)
                    for i in range(2):
                        evac(outs[i][:, sl], ps[i][:], "relu", bias)
            return outs

        # ---------------- c2 conv (64x64 4-tile, windowed) -----------------
        def c2_conv(ins2, ops, dt, bias, tags, wins):
            outs = [npool.tile([128, YB], dt, tag=tg, name=tg) for tg in tags]
            BORD = [0, 2, 4, 6, 1, 3, 5, 7]
            build_wins_full(ins2, wins)
            with tc.tile_pool(name=f"dp{tags[0]}", bufs=2, space="PSUM") as pp:
                for yg in range(8):
                    y0 = 2 * yg
                    ps = [pp.tile([128, 2 * BL], F32, tag=f"p{i}",
                                  name=f"p{i}") for i in range(4)]
                    first = True
                    for dy in (0, -1, 1):
                        n0, N, ysrc = _clip_dy(y0, 2, dy)
                        nys = N // BL
                        for b in BORD:
                            t, hh = b % 4, b // 4
                            nc.tensor.matmul(
                                ps[b // 2][64 * (b % 2):64 * (b % 2) + 64,
                                           n0:n0 + N],
                                ops[dy + 1][t][hh * 64:hh * 64 + 64, :],
                                wins[t][hh * 64:hh * 64 + 64,
                                        ysrc:ysrc + nys, :],
                                start=first, stop=(dy == 1),
                                tile_position=(hh * 64, 64 * (b % 2)))
                        first = False
                    sl = slice(y0 * BL, (y0 + 2) * BL)
                    for i in range(4):
                        evac(outs[i][:, sl], ps[i][:], "relu", bias)
            return outs

        # ---------------- linear (M=img, N=E; returns (img, E) chunks) ----
        def linear(c2o, lw_d, dt, bias_row, tagp, nbufs=8):
            embT = [epool.tile([128, ESZ], F32, tag=f"{tagp}T{m}",
                               name=f"{tagp}T{m}") for m in range(2)]
            with tc.tile_pool(name=f"lw{tagp}", bufs=nbufs) as lwp, \
                 tc.tile_pool(name=f"lp{tagp}", bufs=1, space="PSUM") as pp:
                ps = [pp.tile([128, ESZ], F32, tag=f"p{m}", name=f"p{m}")
                      for m in range(2)]
                for k in range(64):
                    lwt = lwp.tile([128, ESZ], dt, tag="lw", name="lw")
                    nc.gpsimd.dma_start(lwt[:], lw_d[k])
                    cch, y = k // 16, k % 16
                    for m in range(2):
                        lhsT = c2o[cch][:, y * BL + 128 * m:y * BL + 128 * m + 128]
                        nc.tensor.matmul(ps[m][:], lhsT, lwt[:],
                                         start=(k == 0), stop=False)
                for m in range(2):
                    nc.tensor.matmul(ps[m][:], ones_k[:],
                                     bias_row[:], start=False, stop=True)
                for m in range(2):
                    nc.scalar.activation(embT[m][:], ps[m][:], AF.Identity)
            return embT

        def transpose_back(embT, dt, tagp):
            """(img,E) 2 chunks -> (E,img) 4 chunks of dtype dt."""
            emb = [epool.tile([128, BL], dt, tag=f"{tagp}{e}", name=f"{tagp}{e}")
                   for e in range(4)]
            with tc.tile_pool(name=f"tp{tagp}", bufs=2, space="PSUM") as tpp:
                for m in range(2):
                    for e in range(4):
                        tp = tpp.tile([128, 128], F32, tag="tp", name="tp")
                        nc.tensor.transpose(tp[:], embT[m][:, 128 * e:128 * e + 128],
                                            ident_sb[:])
                        nc.vector.tensor_copy(emb[e][:, 128 * m:128 * m + 128],
                                              tp[:])
            return emb

        # ================== tower 2 (fp32 delta path) ==================
        with nc.named_scope("t2emb"):
            d3 = emb_conv(ohd_d, ops_embt2, F32, ["A0", "A1"], winf)
        with nc.named_scope("t2c1"):
            c1o2 = c1_conv(d3, ops_c1t2, F32, bias_sb["b_c1t2"],
                           ["B0", "B1"], winf)
        with nc.named_scope("t2c2"):
            c2o2 = c2_conv(c1o2, ops_c2t2, F32R, bias_sb["b_c2t2"],
                           ["C0", "C1", "A0", "A1"], winf)
        # tower1 emb windows: prefetch on the sync engine while t2lin
        # runs; WAR on the shared window buffers delays this to t2c2-end.
        winb = win_set(BF16)
        for t in range(4):
            nc.sync.dma_start(winb[t][:], ohs_d[t])

        with nc.named_scope("t2lin"):
            embT2 = linear(c2o2, lw2_d, F32R, bl_sb["b_l2"], "e2")
            embed2 = transpose_back(embT2, F32, "e2n")

        # ================== tower 1 (bf16) ==================
        with nc.named_scope("t1emb"):
            se3 = emb_conv(ohs_d, ops_embt1, BF16, ["B0", "B1"], winb,
                           load=False)
        # ================== VQ scores + idx AllGather ==================
        with nc.named_scope("vq"):
            with tc.tile_pool(name="vq", bufs=1) as vqp, \
                 tc.tile_pool(name="vqp", bufs=1, space="PSUM") as vpp:
                znt_sb = []
                for e in range(4):
                    t = vqp.tile([128, NZ], F32, tag=f"znt{e}", name=f"znt{e}")
                    nc.sync.dma_start(t[:], znt_d[128 * e:128 * e + 128, :])
                    znt_sb.append(t)
                sps = [vpp.tile([128, NZ], F32, tag=f"s{m}", name=f"s{m}")
                       for m in range(2)]
                for e in range(4):
                    for m in range(2):
                        nc.tensor.matmul(sps[m][:],
                                         embed2[e][:, 128 * m:128 * m + 128],
                                         znt_sb[e][:], start=(e == 0),
                                         stop=(e == 3))
                for m in range(2):
                    sc = vqp.tile([128, NZ], F32, tag=f"sc{m}", name=f"sc{m}")
                    nc.vector.tensor_copy(sc[:], sps[m][:])
                    mx = vqp.tile([128, 8], F32, tag=f"mx{m}", name=f"mx{m}")
                    nc.vector.max(mx[:], sc[:])
                    ix = vqp.tile([128, 8], U32, tag=f"ix{m}", name=f"ix{m}")
                    nc.vector.max_index(ix[:], mx[:], sc[:])
                    # sync engine: gated on the argmax, but nothing else
                    # needs sync at that point; keeps gpsimd queue moving.
                    nc.sync.dma_start(idx_d[m], ix[:, 0:1])

        with nc.named_scope("t1c1"):
            c1o1 = c1_conv(se3, ops_c1t1, BF16, bias_sb["b_c1t1"],
                           ["C0", "C1"], winb)
        with nc.named_scope("t1c2"):
            c2o1 = c2_conv(c1o1, ops_c2t1, BF16, bias_sb["b_c2t1"],
                           ["A0", "A1", "B0", "B1"], winb)

        # AllGather the 2x128 u32 indices (1KB): triggered late so the
        # gpsimd queue never blocks on the argmax semaphore.
        nc.gpsimd.collective_compute(
            "AllGather", mybir.AluOpType.bypass,
            replica_groups=[list(range(NCORES))],
            ins=[idx_d[:]], outs=[idxg_d[:]])

        with nc.named_scope("t1lin"):
            embT1 = linear(c2o1, lw1_d, BF16, bl_sb["b_l1"], "e1", nbufs=16)
            e1b = transpose_back(embT1, BF16, "e1b")

        # ============== gather all-B codebook rows locally ==============
        # Emitted before t1lin: the gathers + PE transposes overlap the
        # t1 linear phase.  gsb_e reuses dead npool buffers (B*/C* are
        # free once t1c2's matmuls have consumed se3/c1o1).
        with nc.named_scope("zgat"):
            # two (128, 2B) bf16 tiles in the dead C0/C1 buffers; e-chunk
            # view: gsb_e[e] = gt[e//2][:, (e%2)*B : (e%2)*B+B]
            gt = [npool.tile([128, 2 * B], BF16, tag=tg, name=f"gt{j}")
                  for j, tg in enumerate(["C0", "C1"])]
            gsb_e = [gt[e // 2][:, (e % 2) * B:(e % 2) * B + B]
                     for e in range(4)]
            with tc.tile_pool(name="zg", bufs=4) as zgp, \
                 tc.tile_pool(name="zgp", bufs=4, space="PSUM") as zpp:
                gidx = epool.tile([128, 2 * NCORES], U32, tag="gidx",
                                  name="gidx")
                nc.sync.dma_start(gidx[:], idxg_d.rearrange("a p -> p a"))
                for a in range(2 * NCORES):
                    zlg = zgp.tile([128, ESZ], F32, tag="zl", name="zl")
                    nc.gpsimd.indirect_dma_start(
                        out=zlg[:], out_offset=None, in_=zn_d[:],
                        in_offset=bass.IndirectOffsetOnAxis(
                            ap=gidx[:, a:a + 1], axis=0))
                    for e in range(4):
                        tp = zpp.tile([128, 128], F32, tag="tp", name="tp")
                        nc.tensor.transpose(
                            tp[:], zlg[:, 128 * e:128 * e + 128],
                            ident_sb[:])
                        if (a + e) % 2:
                            nc.vector.tensor_copy(
                                gsb_e[e][:, 128 * a:128 * a + 128], tp[:])
                        else:
                            nc.scalar.activation(
                                gsb_e[e][:, 128 * a:128 * a + 128],
                                tp[:], AF.Identity)

        # ================== norms ==================
        with tc.tile_pool(name="nrm", bufs=1) as nrp:
            rnt = [epool.tile([128, 1], F32, tag=f"rnt{m}", name=f"rnt{m}")
                   for m in range(2)]
            for m in range(2):
                sq = nrp.tile([128, ESZ], F32, tag="sq", name="sq")
                nc.vector.tensor_mul(sq[:], embT1[m][:], embT1[m][:])
                n2 = nrp.tile([128, 1], F32, tag="n2", name="n2")
                nc.vector.tensor_reduce(n2[:], sq[:], mybir.AxisListType.X,
                                        mybir.AluOpType.add)
                nc.scalar.sqrt(n2[:], n2[:])
                nc.vector.tensor_scalar_add(n2[:], n2[:], EPS)
                nc.vector.reciprocal(n2[:], n2[:])
                nc.vector.tensor_scalar_mul(rnt[m][:], n2[:], esc)

        # ================== final (bf16) ==================
        with nc.named_scope("fin"):
            with tc.tile_pool(name="finp", bufs=2, space="PSUM") as fpp:
                osb = [npool.tile([128, B], F32, tag=tg, name=f"o{m}")
                       for m, tg in enumerate(["A0", "A1"])]
                for n in range(4):
                    for m in range(2):
                        fp = fpp.tile([128, 512], F32, tag=f"f{m}",
                                      name=f"f{m}")
                        for e in range(4):
                            nc.tensor.matmul(fp[:],
                                             e1b[e][:, 128 * m:128 * m + 128],
                                             gsb_e[e][:, 512 * n:512 * n + 512],
                                             start=(e == 0), stop=(e == 3))
                        nc.vector.tensor_scalar_mul(
                            osb[m][:, 512 * n:512 * n + 512], fp[:], rnt[m][:])
                for m in range(2):
                    nc.sync.dma_start(out_d[128 * m:128 * m + 128, :], osb[m][:])

    nc.compile()
    return nc


def make_in_maps(shared, percore):
    import ml_dtypes
    bf = ml_dtypes.bfloat16

    def b16(x):
        return np.asarray(x, np.float32).astype(bf)

    base = {
        "op_embt1": b16(shared["op_embt1"]),
        "op_embt2": np.ascontiguousarray(shared["op_embt2"], np.float32),
        "op_c1t1": b16(shared["op_c1t1"]),
        "op_c1t2": np.ascontiguousarray(shared["op_c1t2"], np.float32),
        "op_c2t1": b16(shared["op_c2t1"]),
        "op_c2t2": np.ascontiguousarray(shared["op_c2t2"], np.float32),
        "lw1": b16(shared["lw_t1"]),
        "lw2": round_f32r(shared["lw_t2"]),
        "b_c1t1": shared["b_c1t1"], "b_c1t2": shared["b_c1t2"],
        "b_c2t1": shared["b_c2t1"], "b_c2t2": shared["b_c2t2"],
        "b_l1": shared["b_l1"], "b_l2": shared["b_l2"],
        "znt": shared["znT"], "zn": np.ascontiguousarray(shared["znb"],
                                                         np.float32),
        "ident": np.eye(128, dtype=np.float32),
    }
    maps = []
    for pc in percore:
        m = dict(base)
        m["ohs"] = np.ascontiguousarray(pc["ohs"].astype(bf))
        m["ohd"] = np.ascontiguousarray(pc["ohd"])
        maps.append(m)
    return maps


def kernel(**inputs):
    dsf = np.asarray(inputs.get("downscale_factor", 1)).reshape(-1)
    dsf = int(dsf[0]) if dsf.size else 1
    assert dsf == 1, f"only downscale_factor=1 supported, got {dsf}"
    shared, percore, esc, zero_bias = host_prep(inputs)
    nc = build_program(esc, zero_bias)
    maps = make_in_maps(shared, percore)
    res = run_bass_kernel_spmd(nc, maps, list(range(NCORES)))
    out = np.concatenate([res.results[c]["out"] for c in range(NCORES)],
                         axis=0)
    return out.astype(np.float32)


def run_for_test(inputs, trace=False):
    """test.py hook: returns (out, BassKernelResults)."""
    shared, percore, esc, zero_bias = host_prep(inputs)
    nc = build_program(esc, zero_bias)
    maps = make_in_maps(shared, percore)
    res = run_bass_kernel_spmd(nc, maps, list(range(NCORES)), trace=trace)
    out = np.concatenate([res.results[c]["out"] for c in range(NCORES)],
                         axis=0)
    return out.astype(np.float32), res
